# revision 10
# baseline (speedup 1.0000x reference)
"""Trainium2 Bass kernel for the Compressor module (sparse-attention KV
compression): fused  kv/score projections -> overlapped softmax pooling ->
RMSNorm -> RoPE.

Sharding: data-parallel over (batch x seq-half) across 8 cores. Each core
processes 2048 tokens of one batch with a 4-token halo at the front (the
previous compression block), so no collectives are needed. Weights are
replicated.

Layout: matmuls compute out.T = W @ x.T  ([channel, token]), so the
softmax-pooling slots of one compression block are 4 contiguous elements
along the free axis; the "overlap" halves differ only by a 4-token offset
into the same x tile. Pooled results are PE-transposed to [block, channel]
for the RMSNorm / RoPE epilogue and a contiguous output DMA.
"""

import numpy as np

import concourse.bass as bass
import concourse.mybir as mybir
from concourse import bacc
from concourse.tile import TileContext
from concourse.masks import make_identity
from concourse.bass_utils import run_bass_kernel_spmd

B, S, DIM = 4, 4096, 4096
D, RD, RATIO = 512, 64, 4
EPS = 1e-6
NCORES = 8
TOK = 2048          # tokens per core
NCH, CH = 4, 512    # chunks per core, tokens per chunk
NEG = -1.0e30
FP = mybir.dt.float32
MM_DT = mybir.dt.float32r   # full-rate fp32 matmul path on trn2
AX = mybir.AxisListType
ALU = mybir.AluOpType
ACTF = mybir.ActivationFunctionType


def _build_program() -> bass.Bass:
    nc = bacc.Bacc("TRN2", target_bir_lowering=False, debug=False)

    xT = nc.dram_tensor("xt", [DIM, TOK + 4], MM_DT, kind="ExternalInput").ap()
    w = nc.dram_tensor("w", [16, 128, 32, 128], MM_DT,
                       kind="ExternalInput").ap()
    ape = nc.dram_tensor("ape", [8, 128, 4], FP, kind="ExternalInput").ap()
    cosp = nc.dram_tensor("cosp", [512, 32], FP, kind="ExternalInput").ap()
    sinp = nc.dram_tensor("sinp", [512, 32], FP, kind="ExternalInput").ap()
    normb = nc.dram_tensor("normb", [128, 512], FP, kind="ExternalInput").ap()
    scfix = nc.dram_tensor("scfix", [128, 4], FP, kind="ExternalInput").ap()
    out = nc.dram_tensor("out", [512, 512], FP, kind="ExternalOutput").ap()

    with TileContext(nc) as tc:
        with (
            tc.tile_pool(name="const", bufs=1) as constp,
            tc.tile_pool(name="xp", bufs=40) as xp,
            tc.tile_pool(name="wp", bufs=3) as wp,
            tc.tile_pool(name="ep", bufs=4) as ep,
            tc.tile_pool(name="sp", bufs=4) as sp,
            tc.tile_pool(name="yp", bufs=2) as yp,
            tc.tile_pool(name="pmm", bufs=6, space="PSUM") as pmm,
            tc.tile_pool(name="ptr", bufs=2, space="PSUM") as ptr,
        ):
            ident = constp.tile([128, 128], FP)
            make_identity(nc, ident)
            norm_sb = constp.tile([128, 512], FP)
            nc.sync.dma_start(norm_sb, normb)
            fix_sb = constp.tile([128, 4], FP)
            nc.sync.dma_start(fix_sb, scfix)
            ape_sb = constp.tile([128, 8, 4], FP)
            nc.sync.dma_start(ape_sb, ape.rearrange("j p q -> p j q"))
            eps_sb = constp.tile([128, 1], FP)
            nc.gpsimd.memset(eps_sb[:], EPS)

            for ci in range(NCH):
                # x.T chunk: 516 tokens (4-token halo at the front)
                xts = []
                for kt in range(32):
                    t = xp.tile([128, CH + 4], MM_DT, tag="xt")
                    nc.sync.dma_start(
                        t, xT[128 * kt : 128 * kt + 128,
                              CH * ci : CH * ci + CH + 4])
                    xts.append(t)

                cos_sb = sp.tile([128, 32], FP, tag="cos")
                nc.sync.dma_start(cos_sb, cosp[128 * ci : 128 * ci + 128, :])
                sin_sb = sp.tile([128, 32], FP, tag="sin")
                nc.sync.dma_start(sin_sb, sinp[128 * ci : 128 * ci + 128, :])

                y = yp.tile([128, 512], FP, tag="y")

                for j in range(4):
                    wts = {}
                    for oc in (j, j + 4, 8 + j, 12 + j):
                        wt = wp.tile([128, 32, 128], MM_DT, tag="wt")
                        nc.sync.dma_start(wt, w[oc])
                        wts[oc] = wt

                    def mmgroup(oc, off):
                        ps = pmm.tile([128, CH], FP, tag="mmps")
                        for kt in range(32):
                            nc.tensor.matmul(
                                ps,
                                lhsT=wts[oc][:, kt, :],
                                rhs=xts[kt][:, off : off + CH],
                                start=(kt == 0),
                                stop=(kt == 31),
                            )
                        return ps

                    kv_lo = mmgroup(j, 0)        # channels c     (prev block)
                    kv_hi = mmgroup(j + 4, 4)    # channels 512+c (this block)
                    sc_lo = mmgroup(8 + j, 0)
                    sc_hi = mmgroup(12 + j, 4)

                    # score += ape (period-4 along tokens), then exp
                    t_lo = ep.tile([128, CH], FP, tag="et")
                    nc.vector.tensor_tensor(
                        t_lo[:].rearrange("p (b s) -> p b s", s=4),
                        sc_lo[:].rearrange("p (b s) -> p b s", s=4),
                        ape_sb[:, j, None, :].to_broadcast((128, 128, 4)),
                        ALU.add)
                    t_hi = ep.tile([128, CH], FP, tag="et")
                    nc.vector.tensor_tensor(
                        t_hi[:].rearrange("p (b s) -> p b s", s=4),
                        sc_hi[:].rearrange("p (b s) -> p b s", s=4),
                        ape_sb[:, 4 + j, None, :].to_broadcast((128, 128, 4)),
                        ALU.add)
                    if ci == 0:
                        # first block of the shard: -inf fill for the
                        # missing previous block (no-op data on odd cores)
                        nc.vector.tensor_tensor(
                            t_lo[:, 0:4], t_lo[:, 0:4], fix_sb[:], ALU.add)
                    nc.scalar.activation(t_lo[:], t_lo[:], ACTF.Exp)
                    nc.scalar.activation(t_hi[:], t_hi[:], ACTF.Exp)

                    den_a = sp.tile([128, 128], FP, tag="dena")
                    nc.vector.reduce_sum(
                        den_a[:], t_lo[:].rearrange("p (b s) -> p b s", s=4),
                        axis=AX.X)
                    den_b = sp.tile([128, 128], FP, tag="denb")
                    nc.vector.reduce_sum(
                        den_b[:], t_hi[:].rearrange("p (b s) -> p b s", s=4),
                        axis=AX.X)
                    nc.vector.tensor_tensor(den_a[:], den_a[:], den_b[:],
                                            ALU.add)

                    p_lo = ep.tile([128, CH], FP, tag="pt")
                    nc.vector.tensor_tensor(p_lo[:], t_lo[:], kv_lo[:],
                                            ALU.mult)
                    p_hi = ep.tile([128, CH], FP, tag="pt")
                    nc.vector.tensor_tensor(p_hi[:], t_hi[:], kv_hi[:],
                                            ALU.mult)
                    num_a = sp.tile([128, 128], FP, tag="numa")
                    nc.vector.reduce_sum(
                        num_a[:], p_lo[:].rearrange("p (b s) -> p b s", s=4),
                        axis=AX.X)
                    num_b = sp.tile([128, 128], FP, tag="numb")
                    nc.vector.reduce_sum(
                        num_b[:], p_hi[:].rearrange("p (b s) -> p b s", s=4),
                        axis=AX.X)
                    nc.vector.tensor_tensor(num_a[:], num_a[:], num_b[:],
                                            ALU.add)

                    inv = sp.tile([128, 128], FP, tag="inv")
                    nc.vector.reciprocal(inv[:], den_a[:])
                    pooled = sp.tile([128, 128], FP, tag="pooled")
                    nc.vector.tensor_tensor(pooled[:], num_a[:], inv[:],
                                            ALU.mult)

                    # [channel, block] -> [block, channel]
                    trp = ptr.tile([128, 128], FP, tag="trp")
                    nc.tensor.transpose(trp[:], pooled[:], ident[:])
                    nc.scalar.copy(y[:, 128 * j : 128 * j + 128], trp[:])

                # RMSNorm over the 512 channels
                sq = ep.tile([128, CH], FP, tag="sq")
                nc.vector.tensor_tensor(sq[:], y[:], y[:], ALU.mult)
                ssum = sp.tile([128, 1], FP, tag="ssum")
                nc.vector.reduce_sum(ssum[:], sq[:], axis=AX.X)
                rs = sp.tile([128, 1], FP, tag="rs")
                nc.scalar.activation(rs[:], ssum[:], ACTF.Sqrt,
                                     bias=eps_sb[:], scale=1.0 / D)
                inv_rs = sp.tile([128, 1], FP, tag="invrs")
                nc.vector.reciprocal(inv_rs[:], rs[:])
                nc.vector.tensor_scalar_mul(y[:], y[:], inv_rs[:])
                nc.vector.tensor_tensor(y[:], y[:], norm_sb[:], ALU.mult)

                # RoPE on the last 64 channels
                yr = y[:, 448:512].rearrange("p (m two) -> p m two", two=2)
                a, b = yr[:, :, 0], yr[:, :, 1]
                t1 = sp.tile([128, 32], FP, tag="t1")
                t2 = sp.tile([128, 32], FP, tag="t2")
                t3 = sp.tile([128, 32], FP, tag="t3")
                t4 = sp.tile([128, 32], FP, tag="t4")
                nc.vector.tensor_tensor(t1[:], a, cos_sb[:], ALU.mult)
                nc.vector.tensor_tensor(t2[:], b, sin_sb[:], ALU.mult)
                nc.vector.tensor_tensor(t3[:], a, sin_sb[:], ALU.mult)
                nc.vector.tensor_tensor(t4[:], b, cos_sb[:], ALU.mult)
                nc.vector.tensor_tensor(a, t1[:], t2[:], ALU.subtract)
                nc.vector.tensor_tensor(b, t3[:], t4[:], ALU.add)

                nc.sync.dma_start(out[128 * ci : 128 * ci + 128, :], y[:])

    nc.finalize()
    return nc


_PROGRAM = None


def _get_program() -> bass.Bass:
    global _PROGRAM
    if _PROGRAM is None:
        _PROGRAM = _build_program()
    return _PROGRAM


def host_prep(inputs) -> list[dict]:
    x = np.ascontiguousarray(np.asarray(inputs["x"], dtype=np.float32))
    wkv = np.asarray(inputs["wkv_w"], dtype=np.float32)
    wg = np.asarray(inputs["wgate_w"], dtype=np.float32)
    ape = np.asarray(inputs["ape"], dtype=np.float32)
    norm_w = np.asarray(inputs["norm_w"], dtype=np.float32)
    cos = np.asarray(inputs["cos"], dtype=np.float32)
    sin = np.asarray(inputs["sin"], dtype=np.float32)

    W_cat = np.concatenate([wkv, wg], axis=0)          # [2048, 4096]
    # w_prep[oc, ki, kt, m] = W_cat[128*oc + m, 128*kt + ki]
    w_prep = np.ascontiguousarray(
        W_cat.reshape(16, 128, 32, 128).transpose(0, 3, 2, 1))
    ape_prep = np.ascontiguousarray(ape.T.reshape(8, 128, 4))
    cos_s = np.ascontiguousarray(cos[::RATIO][: S // RATIO])   # [1024, 32]
    sin_s = np.ascontiguousarray(sin[::RATIO][: S // RATIO])
    norm_b = np.ascontiguousarray(
        np.broadcast_to(norm_w[None, :], (128, 512)))
    fix_neg = np.full((128, 4), NEG, np.float32)
    fix_zero = np.zeros((128, 4), np.float32)

    in_maps = []
    for c in range(NCORES):
        b, half = c // 2, c % 2
        t0 = half * TOK
        xb = x[b]
        if half == 0:
            xs = np.concatenate(
                [np.zeros((4, DIM), np.float32), xb[:TOK]], axis=0)
        else:
            xs = xb[t0 - 4 : t0 + TOK]
        xT = np.ascontiguousarray(xs.T)                # [4096, 2052]
        in_maps.append(dict(
            xt=xT,
            w=w_prep,
            ape=ape_prep,
            cosp=np.ascontiguousarray(cos_s[half * 512 : half * 512 + 512]),
            sinp=np.ascontiguousarray(sin_s[half * 512 : half * 512 + 512]),
            normb=norm_b,
            scfix=(fix_neg if half == 0 else fix_zero),
        ))
    return in_maps


def assemble(results) -> np.ndarray:
    full = np.zeros((B, S // RATIO, D), np.float32)
    for c in range(NCORES):
        b, half = c // 2, c % 2
        full[b, half * 512 : half * 512 + 512] = results[c]["out"]
    return full


def kernel(**inputs) -> np.ndarray:
    nc = _get_program()
    in_maps = host_prep(inputs)
    res = run_bass_kernel_spmd(nc, in_maps, list(range(NCORES)))
    return assemble(res.results)


# revision 17
# speedup vs baseline: 1.0142x; 1.0142x over previous
"""Trainium2 Bass kernel for the Compressor module (sparse-attention KV
compression): fused  kv/score projections -> overlapped softmax pooling ->
RMSNorm -> RoPE.

Sharding: data-parallel over (batch x seq-half) across 8 cores. Each core
processes 2048 tokens of one batch with a 4-token halo at the front (the
previous compression block), so no collectives are needed. Weights are
replicated.

Layout: matmuls compute out.T = W @ x.T  ([channel, token]), so the
softmax-pooling slots of one compression block are 4 contiguous elements
along the free axis; the "overlap" halves differ only by a 4-token offset
into the same x tile. Pooled results are PE-transposed to [block, channel]
for the RMSNorm / RoPE epilogue and a contiguous output DMA.
"""

import numpy as np

import concourse.bass as bass
import concourse.mybir as mybir
from concourse import bacc
from concourse.tile import TileContext
from concourse.masks import make_identity
from concourse.bass_utils import run_bass_kernel_spmd

B, S, DIM = 4, 4096, 4096
D, RD, RATIO = 512, 64, 4
EPS = 1e-6
NCORES = 8
TOK = 2048          # tokens per core
NCH, CH = 4, 512    # chunks per core, tokens per chunk
NEG = -1.0e30
FP = mybir.dt.float32
MM_DT = mybir.dt.float32r   # full-rate fp32 matmul path on trn2
AX = mybir.AxisListType
ALU = mybir.AluOpType
ACTF = mybir.ActivationFunctionType


def _build_program() -> bass.Bass:
    nc = bacc.Bacc("TRN2", target_bir_lowering=False, debug=False)

    xT = nc.dram_tensor("xt", [DIM, TOK + 4], MM_DT, kind="ExternalInput").ap()
    w = nc.dram_tensor("w", [16, 128, 32, 128], MM_DT,
                       kind="ExternalInput").ap()
    ape = nc.dram_tensor("ape", [8, 128, 4], FP, kind="ExternalInput").ap()
    cosp = nc.dram_tensor("cosp", [512, 32], FP, kind="ExternalInput").ap()
    sinp = nc.dram_tensor("sinp", [512, 32], FP, kind="ExternalInput").ap()
    normb = nc.dram_tensor("normb", [128, 512], FP, kind="ExternalInput").ap()
    scfix = nc.dram_tensor("scfix", [128, 4], FP, kind="ExternalInput").ap()
    out = nc.dram_tensor("out", [512, 512], FP, kind="ExternalOutput").ap()

    KQ = 4   # k-tiles per weight subtile (fine granularity -> DMA queue ||ism)

    with TileContext(nc) as tc:
        with (
            tc.tile_pool(name="const", bufs=1) as constp,
            tc.tile_pool(name="xp", bufs=66) as xp,
            tc.tile_pool(name="wp", bufs=10) as wp,
            tc.tile_pool(name="ep", bufs=5) as ep,
            tc.tile_pool(name="sp", bufs=3) as sp,
            tc.tile_pool(name="yp", bufs=2) as yp,
            tc.tile_pool(name="pmm", bufs=6, space="PSUM") as pmm,
            tc.tile_pool(name="ptr", bufs=2, space="PSUM") as ptr,
        ):
            ident = constp.tile([128, 128], FP)
            make_identity(nc, ident)
            norm_sb = constp.tile([128, 512], FP)
            nc.scalar.dma_start(norm_sb, normb)
            fix_sb = constp.tile([128, 4], FP)
            nc.scalar.dma_start(fix_sb, scfix)
            ape_sb = constp.tile([128, 8, 4], FP)
            nc.scalar.dma_start(ape_sb, ape.rearrange("j p q -> p j q"))
            eps_sb = constp.tile([128, 1], FP)
            nc.gpsimd.memset(eps_sb[:], EPS)

            def load_wsubs(j):
                # weight subtiles for the 4 oc-chunks this j needs, in
                # consumption order (scores first), KQ k-tiles per DMA so
                # loads spread across queues
                subs = {}
                for oc in (8 + j, 12 + j, j, j + 4):
                    subs[oc] = []
                    for q in range(32 // KQ):
                        wt = wp.tile([128, KQ, 128], MM_DT, tag="wt")
                        nc.gpsimd.dma_start(
                            wt, w[oc, :, KQ * q : KQ * q + KQ, :])
                        subs[oc].append(wt)
                return subs

            for pair in range(2):
                # weights for j=0 first so the first matmul group
                # isn't queued behind 17MB of x DMAs
                wsubs0 = load_wsubs(0)

                # x.T chunks: 516 tokens each (4-token halo at the front)
                xts = {}
                for cc in range(2):
                    ci = 2 * pair + cc
                    for kt in range(32):
                        t = xp.tile([128, CH + 4], MM_DT, tag="xt")
                        nc.sync.dma_start(
                            t, xT[128 * kt : 128 * kt + 128,
                                  CH * ci : CH * ci + CH + 4])
                        xts[(cc, kt)] = t

                cos_sb, sin_sb, ys = {}, {}, {}
                for cc in range(2):
                    ci = 2 * pair + cc
                    c_sb = sp.tile([128, 32], FP, tag="cos")
                    nc.scalar.dma_start(c_sb,
                                        cosp[128 * ci : 128 * ci + 128, :])
                    s_sb = sp.tile([128, 32], FP, tag="sin")
                    nc.scalar.dma_start(s_sb,
                                        sinp[128 * ci : 128 * ci + 128, :])
                    cos_sb[cc], sin_sb[cc] = c_sb, s_sb
                    ys[cc] = yp.tile([128, 512], FP, tag="y", name="y")

                for j in range(4):
                    wsubs = wsubs0 if j == 0 else load_wsubs(j)

                    def mmgroup(cc, oc, off):
                        # snake the k order on the 2nd chunk so the
                        # freshest w subtile is reused first and old
                        # subtiles release early
                        ks = (range(32) if cc == 0
                              else range(31, -1, -1))
                        ps = pmm.tile([128, CH], FP, tag="mmps",
                                      name="mmps")
                        for n, kt in enumerate(ks):
                            nc.tensor.matmul(
                                ps,
                                lhsT=wsubs[oc][kt // KQ][:, kt % KQ, :],
                                rhs=xts[(cc, kt)][:, off : off + CH],
                                start=(n == 0),
                                stop=(n == 31),
                            )
                        return ps

                    # scores first: each psum is drained right away by the
                    # ape-add, and the exp/reduce pipeline overlaps the kv
                    # matmul groups that follow
                    e_lo, e_hi, dens = {}, {}, {}
                    sc_ps = {}
                    for cc in range(2):
                        sc_ps[("lo", cc)] = mmgroup(cc, 8 + j, 0)
                    for cc in range(2):
                        sc_ps[("hi", cc)] = mmgroup(cc, 12 + j, 4)
                    for cc in range(2):
                        ci = 2 * pair + cc
                        # score += ape (period-4 along tokens), then exp
                        t_lo = ep.tile([128, CH], FP, tag="et")
                        nc.vector.tensor_tensor(
                            t_lo[:].rearrange("p (b s) -> p b s", s=4),
                            sc_ps[("lo", cc)][:].rearrange(
                                "p (b s) -> p b s", s=4),
                            ape_sb[:, j, None, :].to_broadcast((128, 128, 4)),
                            ALU.add)
                        t_hi = ep.tile([128, CH], FP, tag="et")
                        nc.vector.tensor_tensor(
                            t_hi[:].rearrange("p (b s) -> p b s", s=4),
                            sc_ps[("hi", cc)][:].rearrange(
                                "p (b s) -> p b s", s=4),
                            ape_sb[:, 4 + j, None, :].to_broadcast(
                                (128, 128, 4)),
                            ALU.add)
                        if ci == 0:
                            # first block of the shard: -inf fill for the
                            # missing previous block (no-op on odd cores)
                            nc.vector.tensor_tensor(
                                t_lo[:, 0:4], t_lo[:, 0:4], fix_sb[:],
                                ALU.add)
                        nc.scalar.activation(t_lo[:], t_lo[:], ACTF.Exp)
                        nc.scalar.activation(t_hi[:], t_hi[:], ACTF.Exp)

                        den_a = sp.tile([128, 128], FP, tag="dena")
                        nc.vector.reduce_sum(
                            den_a[:],
                            t_lo[:].rearrange("p (b s) -> p b s", s=4),
                            axis=AX.X)
                        den_b = sp.tile([128, 128], FP, tag="denb")
                        nc.vector.reduce_sum(
                            den_b[:],
                            t_hi[:].rearrange("p (b s) -> p b s", s=4),
                            axis=AX.X)
                        nc.vector.tensor_tensor(den_a[:], den_a[:], den_b[:],
                                                ALU.add)
                        e_lo[cc], e_hi[cc], dens[cc] = t_lo, t_hi, den_a

                    # kv groups: each psum is drained by its weighted
                    # product as soon as the group retires
                    kv_ps = {}
                    for cc in range(2):
                        kv_ps[("lo", cc)] = mmgroup(cc, j, 0)
                    for cc in range(2):
                        kv_ps[("hi", cc)] = mmgroup(cc, j + 4, 4)

                    for cc in range(2):
                        y = ys[cc]
                        p_lo = ep.tile([128, CH], FP, tag="pt")
                        nc.vector.tensor_tensor(p_lo[:], e_lo[cc][:],
                                                kv_ps[("lo", cc)][:],
                                                ALU.mult)
                        p_hi = ep.tile([128, CH], FP, tag="pt")
                        nc.vector.tensor_tensor(p_hi[:], e_hi[cc][:],
                                                kv_ps[("hi", cc)][:],
                                                ALU.mult)
                        num_a = sp.tile([128, 128], FP, tag="numa")
                        nc.vector.reduce_sum(
                            num_a[:],
                            p_lo[:].rearrange("p (b s) -> p b s", s=4),
                            axis=AX.X)
                        num_b = sp.tile([128, 128], FP, tag="numb")
                        nc.vector.reduce_sum(
                            num_b[:],
                            p_hi[:].rearrange("p (b s) -> p b s", s=4),
                            axis=AX.X)
                        nc.vector.tensor_tensor(num_a[:], num_a[:], num_b[:],
                                                ALU.add)

                        inv = sp.tile([128, 128], FP, tag="inv")
                        nc.vector.reciprocal(inv[:], dens[cc][:])
                        pooled = sp.tile([128, 128], FP, tag="pooled")
                        nc.vector.tensor_tensor(pooled[:], num_a[:], inv[:],
                                                ALU.mult)

                        # [channel, block] -> [block, channel]
                        trp = ptr.tile([128, 128], FP, tag="trp")
                        nc.tensor.transpose(trp[:], pooled[:], ident[:])
                        nc.scalar.copy(y[:, 128 * j : 128 * j + 128], trp[:])

                for cc in range(2):
                    ci = 2 * pair + cc
                    y = ys[cc]

                    # RMSNorm over the 512 channels
                    sq = ep.tile([128, CH], FP, tag="pt")
                    nc.vector.tensor_tensor(sq[:], y[:], y[:], ALU.mult)
                    ssum = sp.tile([128, 1], FP, tag="ssum")
                    nc.vector.reduce_sum(ssum[:], sq[:], axis=AX.X)
                    rs = sp.tile([128, 1], FP, tag="rs")
                    nc.scalar.activation(rs[:], ssum[:], ACTF.Sqrt,
                                         bias=eps_sb[:], scale=1.0 / D)
                    inv_rs = sp.tile([128, 1], FP, tag="invrs")
                    nc.vector.reciprocal(inv_rs[:], rs[:])
                    nc.vector.tensor_scalar_mul(y[:], y[:], inv_rs[:])
                    nc.vector.tensor_tensor(y[:], y[:], norm_sb[:], ALU.mult)

                    # RoPE on the last 64 channels
                    yr = y[:, 448:512].rearrange("p (m two) -> p m two",
                                                 two=2)
                    a, b = yr[:, :, 0], yr[:, :, 1]
                    t1 = sp.tile([128, 32], FP, tag="t1")
                    t2 = sp.tile([128, 32], FP, tag="t2")
                    t3 = sp.tile([128, 32], FP, tag="t3")
                    t4 = sp.tile([128, 32], FP, tag="t4")
                    nc.vector.tensor_tensor(t1[:], a, cos_sb[cc][:], ALU.mult)
                    nc.vector.tensor_tensor(t2[:], b, sin_sb[cc][:], ALU.mult)
                    nc.vector.tensor_tensor(t3[:], a, sin_sb[cc][:], ALU.mult)
                    nc.vector.tensor_tensor(t4[:], b, cos_sb[cc][:], ALU.mult)
                    nc.vector.tensor_tensor(a, t1[:], t2[:], ALU.subtract)
                    nc.vector.tensor_tensor(b, t3[:], t4[:], ALU.add)

                    nc.scalar.dma_start(out[128 * ci : 128 * ci + 128, :],
                                        y[:])

    nc.finalize()
    return nc


_PROGRAM = None


def _get_program() -> bass.Bass:
    global _PROGRAM
    if _PROGRAM is None:
        _PROGRAM = _build_program()
    return _PROGRAM


def host_prep(inputs) -> list[dict]:
    x = np.ascontiguousarray(np.asarray(inputs["x"], dtype=np.float32))
    wkv = np.asarray(inputs["wkv_w"], dtype=np.float32)
    wg = np.asarray(inputs["wgate_w"], dtype=np.float32)
    ape = np.asarray(inputs["ape"], dtype=np.float32)
    norm_w = np.asarray(inputs["norm_w"], dtype=np.float32)
    cos = np.asarray(inputs["cos"], dtype=np.float32)
    sin = np.asarray(inputs["sin"], dtype=np.float32)

    W_cat = np.concatenate([wkv, wg], axis=0)          # [2048, 4096]
    # w_prep[oc, ki, kt, m] = W_cat[128*oc + m, 128*kt + ki]
    w_prep = np.ascontiguousarray(
        W_cat.reshape(16, 128, 32, 128).transpose(0, 3, 2, 1))
    ape_prep = np.ascontiguousarray(ape.T.reshape(8, 128, 4))
    cos_s = np.ascontiguousarray(cos[::RATIO][: S // RATIO])   # [1024, 32]
    sin_s = np.ascontiguousarray(sin[::RATIO][: S // RATIO])
    norm_b = np.ascontiguousarray(
        np.broadcast_to(norm_w[None, :], (128, 512)))
    fix_neg = np.full((128, 4), NEG, np.float32)
    fix_zero = np.zeros((128, 4), np.float32)

    in_maps = []
    for c in range(NCORES):
        b, half = c // 2, c % 2
        t0 = half * TOK
        xb = x[b]
        if half == 0:
            xs = np.concatenate(
                [np.zeros((4, DIM), np.float32), xb[:TOK]], axis=0)
        else:
            xs = xb[t0 - 4 : t0 + TOK]
        xT = np.ascontiguousarray(xs.T)                # [4096, 2052]
        in_maps.append(dict(
            xt=xT,
            w=w_prep,
            ape=ape_prep,
            cosp=np.ascontiguousarray(cos_s[half * 512 : half * 512 + 512]),
            sinp=np.ascontiguousarray(sin_s[half * 512 : half * 512 + 512]),
            normb=norm_b,
            scfix=(fix_neg if half == 0 else fix_zero),
        ))
    return in_maps


def assemble(results) -> np.ndarray:
    full = np.zeros((B, S // RATIO, D), np.float32)
    for c in range(NCORES):
        b, half = c // 2, c % 2
        full[b, half * 512 : half * 512 + 512] = results[c]["out"]
    return full


def kernel(**inputs) -> np.ndarray:
    nc = _get_program()
    in_maps = host_prep(inputs)
    res = run_bass_kernel_spmd(nc, in_maps, list(range(NCORES)))
    return assemble(res.results)
